# revision 6
# baseline (speedup 1.0000x reference)
"""Trainium2 Bass kernel v2 for a 2-layer GCN (PyG GCNConv semantics).

v3 architecture (vs v2):
  - Layer 1 needs only input data (x) per edge, so the host pre-expands the
    dinv-prescaled x into edge-stream order (im2col-style layout transform).
    The device streams it SEQUENTIALLY at full HBM rate -- zero gather
    packets for layer 1.  Aggregation, W1 transform, and relu all on device.
  - Layer 2 is the runtime halo exchange: AllGather h (sectioned A/B), SWDGE
    gather of 256B h rows from DRAM (4 queues, big descriptor carveout, with
    the SDMA engines now all to itself), feat-major aggregation, dense W2.

Math:  h   = relu(dinv_dst * ((sum_e S dinv_src x[src]) @ W1) + b1)
       out =       dinv_dst * ((sum_e S dinv_src h[src]) @ W2) + b2
with S host-built one-hot (exact fp8 1.0), deg counted at target incl self-loop.
"""

import numpy as np
import ml_dtypes

BF16 = ml_dtypes.bfloat16
FP8 = ml_dtypes.float8_e4m3fn

# ---- hardcoded problem constants ----
N_FULL = 50000
F_IN = 256
HID = 128
OUT = 64
NCORES = 8
P = 128
CHUNK = 24         # L2 gather-call granularity in 128-edge tiles
CHUNK1 = 16        # L1 sequential-stream slab in tiles
GROUP = 4          # L1 dst-block group size (A/B runs interleaved per group)
NQUEUES = 4
DMA_SCRATCH = 49152  # 3072-descriptor carveout per SWDGE queue


# ----------------------------------------------------------------------------
# host-side graph preprocessing
# ----------------------------------------------------------------------------

def _ceil(a, b):
    return -(-a // b)


def _balance_nodes(n, nshard, nblk, weight):
    """LPT-greedy node -> (core, block, slot) assignment balancing per-block
    aggregation work.  Returns perm (block-concatenated order) and pos."""
    import heapq

    lastcap = nshard - (nblk - 1) * P
    nb = NCORES * nblk
    caps = np.full(nb, P, np.int64)
    caps[nblk - 1:: nblk] = lastcap
    order = np.argsort(-weight, kind="stable")
    heap = [(0.0, int(b)) for b in range(nb)]
    heapq.heapify(heap)
    members = [[] for _ in range(nb)]
    for node in order:
        while True:
            w, b = heapq.heappop(heap)
            if len(members[b]) < caps[b]:
                members[b].append(node)
                if len(members[b]) < caps[b]:
                    heapq.heappush(heap, (w + weight[node], b))
                break
    perm = np.concatenate([np.asarray(m, np.int64) for m in members])
    pos = np.empty(n, np.int64)
    pos[perm] = np.arange(n)
    return perm, pos


def _streams(counts, tiles_hb, nblk, order_kind, group):
    """Build a tile stream.  order_kind 'group': per block-group, the half-0
    tiles of its blocks then the half-1 tiles (psum accumulates both halves
    consecutively per block).  order_kind 'half': all half-0 tiles then all
    half-1 (v1 order, needs acc bridging).  Returns tile list
    [(h, b, first, last)] and chunks [(h, stream_start, cnt)]."""
    tiles = []
    if order_kind == "group":
        for g0 in range(0, nblk, group):
            bs = range(g0, min(g0 + group, nblk))
            for h in (0, 1):
                for b in bs:
                    for i in range(int(tiles_hb[h, b])):
                        tiles.append([h, b, False, False])
    else:
        for h in (0, 1):
            for b in range(nblk):
                for i in range(int(tiles_hb[h, b])):
                    tiles.append([h, b, False, False])
    # 'group': psum spans both halves of a block -> flags per block.
    # 'half': psum per (half, block), bridged via acc -> flags per (h, b).
    seen_first = set()
    last_idx = {}
    for i, (h, b, _, _) in enumerate(tiles):
        k = b if order_kind == "group" else (h, b)
        if k not in seen_first:
            tiles[i][2] = True
            seen_first.add(k)
        last_idx[k] = i
    for k, i in last_idx.items():
        tiles[i][3] = True
    # chunks: split each same-half run into near-equal pieces <= CHUNK tiles
    chunks = []
    i = 0
    while i < len(tiles):
        h = tiles[i][0]
        j = i
        while j < len(tiles) and tiles[j][0] == h:
            j += 1
        run = j - i
        nch = _ceil(run, CHUNK)
        base, rem = divmod(run, nch)
        s = i
        for k in range(nch):
            c = base + (1 if k < rem else 0)
            chunks.append((h, s, c))
            s += c
        i = j
    return [tuple(t) for t in tiles], chunks


def host_prep(edge_index, n):
    nshard = n // NCORES
    nblk = _ceil(nshard, P)

    src = np.asarray(edge_index[0], np.int64)
    dst = np.asarray(edge_index[1], np.int64)
    loops = np.arange(n, dtype=np.int64)
    src_a = np.concatenate([src, loops])
    dst_a = np.concatenate([dst, loops])
    deg = np.bincount(dst_a, minlength=n).astype(np.float64)  # incl self-loop
    dinv = 1.0 / np.sqrt(deg)

    perm, pos = _balance_nodes(n, nshard, nblk, deg)

    secb = 20
    sa = secb * P                 # section-A rows per core (local)
    sbw = nblk * P - sa           # section-B padded rows per core
    psrc = pos[src_a]
    pdst = pos[dst_a]
    core = pdst // nshard
    dstl = pdst - core * nshard
    blk = dstl // P
    slot = dstl - blk * P
    src_core = psrc // nshard
    srcl = psrc - src_core * nshard
    sect = (srcl >= sa).astype(np.int64)
    tidx = np.where(sect == 0, src_core * sa + srcl, src_core * sbw + (srcl - sa))
    assert tidx.max() < 32768
    half = sect

    E = src.shape[0]
    noself = np.ones(src_a.shape[0], bool)
    noself[E:] = False            # appended self-loops: L1 only
    key = (core * 2 + half) * nblk + blk
    counts = np.bincount(key, minlength=NCORES * 2 * nblk).reshape(NCORES, 2, nblk)
    key2m = key[noself]
    counts2 = np.bincount(key2m, minlength=NCORES * 2 * nblk).reshape(NCORES, 2, nblk)
    tiles_hb = np.ceil(counts2.max(axis=0) / P).astype(np.int64)  # [2, nblk] (L2)
    assert (counts2.max(axis=0) > 0).all(), "empty (half, block) group in L2"

    order = np.argsort(key, kind="stable")
    s_src = tidx[order]
    s_slot = slot[order]
    goff = np.zeros(NCORES * 2 * nblk + 1, np.int64)
    np.cumsum(counts.reshape(-1), out=goff[1:])

    order2 = np.argsort(key2m, kind="stable")
    s2_src = tidx[noself][order2]
    s2_slot = slot[noself][order2]
    goff2 = np.zeros(NCORES * 2 * nblk + 1, np.int64)
    np.cumsum(counts2.reshape(-1), out=goff2[1:])

    # L1: merged halves (no gather -> no int16 constraint), plain block order
    counts1 = counts.sum(axis=1)                      # [NCORES, nblk]  incl self
    tiles_b = np.ceil(counts1.max(axis=0) / P).astype(np.int64)  # [nblk]
    tiles1 = []
    for b in range(nblk):
        for i in range(int(tiles_b[b])):
            tiles1.append((0, b, i == 0, i == int(tiles_b[b]) - 1))
    t1_total = len(tiles1)
    chunks1 = []
    s = 0
    while s < t1_total:
        c = min(CHUNK1, t1_total - s)
        chunks1.append((0, s, c))
        s += c

    tiles2, chunks2 = _streams(counts, tiles_hb, nblk, "half", GROUP)
    t_total = len(tiles2)

    def build_stream(tiles):
        """Per-core idx (wrapped int16) + one-hot smat for a tile stream."""
        # stream position of i-th tile of (h, b)
        occ = {}
        posmap = {}
        for i, (h, b, _, _) in enumerate(tiles):
            k = occ.get((h, b), 0)
            posmap[(h, b, k)] = i
            occ[(h, b)] = k + 1
        idx_flat = np.zeros((NCORES, t_total * P), np.int16)
        slot_flat = np.zeros((NCORES, t_total * P), np.int64)
        fill = np.zeros((NCORES, t_total * P), bool)
        for c in range(NCORES):
            for h in (0, 1):
                for b in range(nblk):
                    g0 = goff2[(c * 2 + h) * nblk + b]
                    cnt = int(counts2[c, h, b])
                    for k in range(int(tiles_hb[h, b])):
                        tp = posmap[(h, b, k)]
                        lo = k * P
                        m = min(P, cnt - lo)
                        if m <= 0:
                            break
                        o0 = tp * P
                        idx_flat[c, o0: o0 + m] = s2_src[g0 + lo: g0 + lo + m].astype(np.int16)
                        slot_flat[c, o0: o0 + m] = s2_slot[g0 + lo: g0 + lo + m]
                        fill[c, o0: o0 + m] = True
        idx_w = np.empty((NCORES, 128, t_total * 8), np.int16)
        smat = np.zeros((NCORES, 128, t_total, 128), FP8)
        ar = np.arange(t_total * P)
        for c in range(NCORES):
            w = idx_flat[c].reshape(-1, 16).T
            idx_w[c] = np.tile(w, (8, 1))
            nz = fill[c]
            smat[c, ar[nz] % P, ar[nz] // P, slot_flat[c][nz]] = FP8(1.0)
        return idx_w, smat.reshape(NCORES, 128, t_total * 128)

    idx2, smat2 = build_stream(tiles2)

    # L1 host-side: per-core edge-order source row ids (into padded perm
    # order) + smat, in block-major tile order
    order1 = np.argsort(core * nblk + blk, kind="stable")
    s1_psrc = psrc[order1]
    s1_slot = slot[order1]
    goff1 = np.zeros(NCORES * nblk + 1, np.int64)
    np.cumsum(counts1.reshape(-1), out=goff1[1:])
    esrc = np.zeros((NCORES, t1_total * P), np.int64)   # padded-perm src pos
    efill = np.zeros((NCORES, t1_total * P), bool)
    eslot = np.zeros((NCORES, t1_total * P), np.int64)
    tstart1 = np.zeros(nblk, np.int64)
    np.cumsum(tiles_b[:-1], out=tstart1[1:])
    for c in range(NCORES):
        for b in range(nblk):
            g0 = goff1[c * nblk + b]
            cnt = int(counts1[c, b])
            o0 = tstart1[b] * P
            esrc[c, o0: o0 + cnt] = s1_psrc[g0: g0 + cnt]
            eslot[c, o0: o0 + cnt] = s1_slot[g0: g0 + cnt]
            efill[c, o0: o0 + cnt] = True
    smat1 = np.zeros((NCORES, 128, t1_total, 128), FP8)
    ar1 = np.arange(t1_total * P)
    for c in range(NCORES):
        nz = efill[c]
        smat1[c, ar1[nz] % P, ar1[nz] // P, eslot[c][nz]] = FP8(1.0)
    smat1 = smat1.reshape(NCORES, 128, t1_total * 128)

    # per-(core, slot, blk) dinv of resident node (0 on pad slots)
    dinv_blk = np.zeros((NCORES, P, nblk), np.float32)
    for c in range(NCORES):
        for b in range(nblk):
            v = nshard - b * P if b == nblk - 1 else P
            nodes = perm[c * nshard + b * P: c * nshard + b * P + v]
            dinv_blk[c, :v, b] = dinv[nodes]

    st = dict(
        nshard=nshard, nblk=nblk, lastv=nshard - (nblk - 1) * P,
        t_total=t_total, t1_total=t1_total, tiles_hb=tiles_hb,
        tiles1=tiles1, chunks1=chunks1, tiles2=tiles2, chunks2=chunks2,
        secb=secb, sa=sa, sbw=sbw,
    )
    percore = dict(esrc=esrc, smat1=smat1, idx2=idx2, smat2=smat2,
                   dinv_blk=dinv_blk)
    return st, percore, perm, pos, dinv


def build_xedge(x, perm, dinv, st, esrc, efill=None):
    """Per-core edge-stream x: xedge[c][p, t, f] = dinv[src]*x[src] of edge
    slot p of tile t (bf16, zeros on pad slots)."""
    t1 = st["t1_total"]
    xs = (x * dinv[:, None].astype(np.float32)).astype(BF16)
    xsp = xs[perm]  # padded-perm order == psrc indexing (unpadded, n rows)
    out = []
    for c in range(NCORES):
        xe = xsp[esrc[c]]                     # [t1*128, f_in]
        xe = xe.reshape(t1, P, x.shape[1]).transpose(1, 0, 2)
        out.append(np.ascontiguousarray(xe))
    return out


# ----------------------------------------------------------------------------
# device program
# ----------------------------------------------------------------------------

def build_program(st, f_in, hid, out_f, has_b1, has_b2, enable_asserts=False):
    import concourse.mybir as mybir
    import concourse.tile as tile
    from concourse import bacc

    dt = mybir.dt
    Alu = mybir.AluOpType
    Act = mybir.ActivationFunctionType

    nshard, nblk, lastv = st["nshard"], st["nblk"], st["lastv"]
    t_total = st["t_total"]
    tiles_hb = st["tiles_hb"]
    secb, sa, sbw = st["secb"], st["sa"], st["sbw"]
    kt = f_in // P

    nc = bacc.Bacc(
        "TRN2", target_bir_lowering=False, debug=False,
        enable_asserts=enable_asserts, num_devices=NCORES,
        num_swdge_queues=NQUEUES, dynamic_dma_scratch_size=DMA_SCRATCH,
    )

    # ---- I/O ----
    t1_total = st["t1_total"]
    xe_d = nc.dram_tensor("xedge", [P, t1_total, f_in], dt.bfloat16, kind="ExternalInput")
    w1_d = nc.dram_tensor("w1", [P, kt, hid], dt.bfloat16, kind="ExternalInput")
    w2_d = nc.dram_tensor("w2", [hid, out_f], dt.bfloat16, kind="ExternalInput")
    eye_d = nc.dram_tensor("eye", [P, P], dt.bfloat16, kind="ExternalInput")
    idx2_d = nc.dram_tensor("idx2", [128, t_total * 8], dt.int16, kind="ExternalInput")
    smat1_d = nc.dram_tensor("smat1", [128, t1_total * 128], dt.float8e4, kind="ExternalInput")
    smat2_d = nc.dram_tensor("smat2", [128, t_total * 128], dt.float8e4, kind="ExternalInput")
    dinv_d = nc.dram_tensor("dinv", [P, nblk], dt.float32, kind="ExternalInput")
    if has_b1:
        b1_d = nc.dram_tensor("b1bc", [P, hid], dt.float32, kind="ExternalInput")
    if has_b2:
        b2_d = nc.dram_tensor("b2bc", [P, out_f], dt.float32, kind="ExternalInput")
    out_d = nc.dram_tensor("out", [nshard, out_f], dt.float32, kind="ExternalOutput")

    rg = [list(range(NCORES))]

    def bts(i, sz):
        return slice(i * sz, (i + 1) * sz)

    with tile.TileContext(nc) as tc:
        with (
            tc.tile_pool(name="const", bufs=1) as constp,
            tc.tile_pool(name="stage", bufs=1) as stagep,
            tc.tile_pool(name="dram", bufs=1, space="DRAM") as dramp,
            tc.tile_pool(name="g1pool", bufs=3) as g1pool,
            tc.tile_pool(name="g2pool", bufs=6) as g2pool,
            tc.tile_pool(name="s1pool", bufs=3) as s1pool,
            tc.tile_pool(name="s2pool", bufs=6) as s2pool,
            tc.tile_pool(name="epool", bufs=3) as epool,
        ):
            w1_sb = constp.tile([P, kt, hid], dt.bfloat16)
            w2_sb = constp.tile([hid, out_f], dt.bfloat16)
            eye_sb = constp.tile([P, P], dt.bfloat16)
            dinv_sb = constp.tile([P, nblk], dt.float32)
            nc.sync.dma_start(out=w1_sb[:], in_=w1_d[:])
            nc.sync.dma_start(out=w2_sb[:], in_=w2_d[:])
            nc.sync.dma_start(out=eye_sb[:], in_=eye_d[:])
            nc.sync.dma_start(out=dinv_sb[:], in_=dinv_d[:])
            dinv2_sb = constp.tile([P, nblk], dt.float32)
            nc.vector.tensor_tensor(out=dinv2_sb[:], in0=dinv_sb[:], in1=dinv_sb[:], op=Alu.mult)
            if has_b1:
                b1_sb = constp.tile([P, hid], dt.float32)
                nc.sync.dma_start(out=b1_sb[:], in_=b1_d[:])
            if has_b2:
                b2_sb = constp.tile([P, out_f], dt.float32)
                nc.sync.dma_start(out=b2_sb[:], in_=b2_d[:])

            idx2_sb = constp.tile([128, st["t_total"] * 8], dt.int16)
            nc.sync.dma_start(out=idx2_sb[:], in_=idx2_d[:])

            # tensor warmup burst: release the HAM clock throttle early
            with tc.tile_pool(name="pwarm", bufs=1, space="PSUM") as pwarm:
                wps = pwarm.tile([P, P], dt.float32)
                for _w in range(48):
                    nc.tensor.matmul(out=wps[:], lhsT=eye_sb[:], rhs=eye_sb[:],
                                     start=(_w == 0), stop=(_w == 47))

            hstage = stagep.tile([P, nblk, hid], dt.bfloat16)
            acc = stagep.tile([P, nblk * hid], dt.float32)   # L2 slot-major acc
            outstage = stagep.tile([P, nblk * out_f], dt.float32)

            h_loc = dramp.tile([nblk * P, hid], dt.bfloat16, name="h_loc")
            hA_full = dramp.tile([NCORES * sa, hid], dt.bfloat16, addr_space="Shared",
                                 name="hA_full")
            hB_full = dramp.tile([NCORES * sbw, hid], dt.bfloat16, addr_space="Shared",
                                 name="hB_full")

            # ================= Layer 1: aggregate-first in x-space =========
            with (
                tc.tile_pool(name="pagg1", bufs=4, space="PSUM") as pagg1,
                tc.tile_pool(name="pT", bufs=2, space="PSUM") as pTp,
                tc.tile_pool(name="pH", bufs=2, space="PSUM") as pHp,
            ):
                psd = {}
                ecnt = 0
                for ci, (h, cstart, cnt) in enumerate(st["chunks1"]):
                    g = g1pool.tile([P, CHUNK1, f_in], dt.bfloat16, tag="g1")
                    nc.sync.dma_start(out=g[:, :cnt, :],
                                      in_=xe_d[:, cstart:cstart + cnt, :])
                    s_sb = s1pool.tile([P, CHUNK1, P], dt.float8e4, tag="s1")
                    nc.scalar.dma_start(out=s_sb[:, :cnt, :],
                                        in_=smat1_d[:, cstart * 128:(cstart + cnt) * 128])
                    for p in range(cnt):
                        t = cstart + p
                        th, b, first, last = st["tiles1"][t]
                        if first:
                            psd[b] = pagg1.tile([P, f_in], dt.float32, tag="ps1", name="ps1")
                        nc.tensor.matmul(out=psd[b][:], lhsT=s_sb[:, p, :], rhs=g[:, p, :],
                                         start=first, stop=last)
                        if last:
                            # block epilogue: aggx -> transpose -> @W1 -> relu
                            aggx = epool.tile([P, f_in], dt.bfloat16, tag="aggx")
                            cp_eng = nc.scalar if b % 2 == 0 else nc.vector
                            if b % 2 == 0:
                                nc.scalar.activation(out=aggx[:], in_=psd[b][:], func=Act.Copy)
                            else:
                                nc.vector.tensor_copy(out=aggx[:], in_=psd[b][:])
                            del psd[b]
                            pT = pTp.tile([P, kt, P], dt.float32, tag="pT")
                            for k in range(kt):
                                nc.tensor.matmul(out=pT[:, k, :], lhsT=aggx[:, bts(k, P)],
                                                 rhs=eye_sb[:], start=True, stop=True)
                            aggxT = epool.tile([P, kt, P], dt.bfloat16, tag="aggxT")
                            if b % 2 == 0:
                                nc.vector.tensor_copy(out=aggxT[:], in_=pT[:])
                            else:
                                nc.scalar.activation(
                                    out=aggxT[:].rearrange("p a b -> p (a b)"),
                                    in_=pT[:].rearrange("p a b -> p (a b)"), func=Act.Copy)
                            pH = pHp.tile([P, hid], dt.float32, tag="pH")
                            for k in range(kt):
                                nc.tensor.matmul(out=pH[:], lhsT=aggxT[:, k, :],
                                                 rhs=w1_sb[:, k, :],
                                                 start=(k == 0), stop=(k == kt - 1))
                            if has_b1:
                                tmp = epool.tile([P, hid], dt.float32, tag="tmp1")
                                nc.vector.tensor_scalar(out=tmp[:], in0=pH[:],
                                                        scalar1=dinv_sb[:, b: b + 1], op0=Alu.mult)
                                nc.vector.tensor_tensor(out=tmp[:], in0=tmp[:], in1=b1_sb[:],
                                                        op=Alu.add)
                                nc.scalar.activation(out=hstage[:, b, :], in_=tmp[:],
                                                     func=Act.Relu, scale=dinv_sb[:, b: b + 1])
                            else:
                                nc.scalar.activation(out=hstage[:, b, :], in_=pH[:],
                                                     func=Act.Relu, scale=dinv2_sb[:, b: b + 1])
                            ecnt += 1
                            if ecnt == secb:
                                nc.scalar.dma_start(
                                    out=h_loc[:sa].rearrange("(b s) f -> s b f", s=P),
                                    in_=hstage[:, :secb, :])
                nc.scalar.dma_start(
                    out=h_loc[sa:].rearrange("(b s) f -> s b f", s=P),
                    in_=hstage[:, secb:, :])

            nc.gpsimd.collective_compute(
                "AllGather", mybir.AluOpType.bypass, replica_groups=rg,
                ins=[h_loc[:sa]], outs=[hA_full[:]])
            nc.gpsimd.collective_compute(
                "AllGather", mybir.AluOpType.bypass, replica_groups=rg,
                ins=[h_loc[sa:]], outs=[hB_full[:]])

            # ========= Layer 2: slot-major agg (fp8 stationary) + W2 =======
            with (
                tc.tile_pool(name="pagg2", bufs=6, space="PSUM") as pagg2,
                tc.tile_pool(name="pT2", bufs=1, space="PSUM") as pT2p,
                tc.tile_pool(name="pD", bufs=1, space="PSUM") as pD,
            ):
                psd2 = {}
                for ci, (h, cstart, cnt) in enumerate(st["chunks2"]):
                    g = g2pool.tile([P, CHUNK, hid], dt.bfloat16, tag="g2")
                    src_ap = hA_full[:] if h == 0 else hB_full[:]
                    nc.gpsimd.dma_gather(
                        g[:, :cnt, :], src_ap,
                        idx2_sb[:, cstart * 8:(cstart + cnt) * 8],
                        cnt * P, cnt * P, hid, single_packet=False,
                        queue_num=ci % NQUEUES,
                    )
                    s_sb = s2pool.tile([P, CHUNK, P], dt.float8e4, tag="s2")
                    s_eng = nc.sync if ci % 2 == 0 else nc.scalar
                    s_eng.dma_start(out=s_sb[:, :cnt, :],
                                    in_=smat2_d[:, cstart * 128:(cstart + cnt) * 128])
                    for p in range(cnt):
                        t = cstart + p
                        th, b, first, last = st["tiles2"][t]
                        if first:
                            psd2[b] = pagg2.tile([P, P], dt.float32, tag="ps2", name="ps2")
                        nc.tensor.matmul(out=psd2[b][:], lhsT=s_sb[:, p, :], rhs=g[:, p, :],
                                         start=first, stop=last)
                        if not last:
                            continue
                        if th == 0 and tiles_hb[1, b] > 0:
                            # fold the self-loop term (hstage rows already carry
                            # the dinv_src prescale -> add directly)
                            nc.vector.tensor_tensor(out=acc[:, bts(b, P)], in0=psd2[b][:],
                                                    in1=hstage[:, b, :], op=Alu.add)
                            del psd2[b]
                            continue
                        # final half for this block: agg[slot, f] -> out
                        aggS = epool.tile([P, P], dt.bfloat16, tag="aggS")
                        if tiles_hb[0, b] > 0 and th == 1:
                            nc.vector.tensor_tensor(out=aggS[:], in0=psd2[b][:],
                                                    in1=acc[:, bts(b, P)], op=Alu.add)
                        else:
                            nc.vector.tensor_tensor(out=aggS[:], in0=psd2[b][:],
                                                    in1=hstage[:, b, :], op=Alu.add)
                        del psd2[b]
                        pT2 = pT2p.tile([P, P], dt.float32, tag="pT2", name="pT2")
                        nc.tensor.matmul(out=pT2[:], lhsT=aggS[:], rhs=eye_sb[:],
                                         start=True, stop=True)
                        aggF = epool.tile([P, P], dt.bfloat16, tag="aggF")
                        nc.scalar.activation(out=aggF[:], in_=pT2[:], func=Act.Copy)
                        ps = pD.tile([P, out_f], dt.float32, tag="psD", name="psD")
                        nc.tensor.matmul(out=ps[:], lhsT=aggF[:], rhs=w2_sb[:],
                                         start=True, stop=True)
                        if has_b2:
                            nc.vector.tensor_scalar(out=outstage[:, bts(b, out_f)], in0=ps[:],
                                                    scalar1=dinv_sb[:, b: b + 1], op0=Alu.mult)
                            nc.vector.tensor_tensor(out=outstage[:, bts(b, out_f)],
                                                    in0=outstage[:, bts(b, out_f)],
                                                    in1=b2_sb[:], op=Alu.add)
                        else:
                            nc.scalar.activation(out=outstage[:, bts(b, out_f)], in_=ps[:],
                                                 func=Act.Copy, scale=dinv_sb[:, b: b + 1])
                nc.scalar.dma_start(
                    out=out_d[: (nblk - 1) * P, :].rearrange("(b s) f -> s b f", s=P),
                    in_=outstage[:, : (nblk - 1) * out_f].rearrange("s (b f) -> s b f", f=out_f))
                nc.scalar.dma_start(
                    out=out_d[(nblk - 1) * P:, :],
                    in_=outstage[:lastv, bts(nblk - 1, out_f)])


    nc.compile()
    return nc


# ----------------------------------------------------------------------------
# input packing / entry point
# ----------------------------------------------------------------------------

_CACHE = {}


def _run(x, edge_index, W1, b1, W2, b2, trace=False):
    from concourse.bass_utils import run_bass_kernel_spmd

    n = x.shape[0]
    f_in = x.shape[1]
    hid = W1.shape[1]
    out_f = W2.shape[1]
    has_b1 = bool(np.any(b1))
    has_b2 = bool(np.any(b2))

    st, percore, perm, pos, dinv = host_prep(edge_index, n)
    xedges = build_xedge(x, perm, dinv, st, percore["esrc"])

    kt = f_in // P
    w1h = np.ascontiguousarray(W1.reshape(kt, P, hid).transpose(1, 0, 2)).astype(BF16)
    w2h = np.ascontiguousarray(W2).astype(BF16)
    eye = np.eye(P, dtype=np.float32).astype(BF16)

    key = (n, f_in, hid, out_f, st["t_total"], tuple(st["tiles_hb"].reshape(-1)),
           has_b1, has_b2)
    nc = _CACHE.get(key)
    if nc is None:
        nc = build_program(st, f_in, hid, out_f, has_b1, has_b2)
        _CACHE[key] = nc

    in_maps = []
    for c in range(NCORES):
        m = {
            "xedge": xedges[c], "w1": w1h, "w2": w2h, "eye": eye,
            "idx2": np.ascontiguousarray(percore["idx2"][c]),
            "smat1": np.ascontiguousarray(percore["smat1"][c]),
            "smat2": np.ascontiguousarray(percore["smat2"][c]),
            "dinv": np.ascontiguousarray(percore["dinv_blk"][c]),
        }
        if has_b1:
            m["b1bc"] = np.ascontiguousarray(np.broadcast_to(b1, (P, hid))).astype(np.float32)
        if has_b2:
            m["b2bc"] = np.ascontiguousarray(np.broadcast_to(b2, (P, out_f))).astype(np.float32)
        in_maps.append(m)

    res = run_bass_kernel_spmd(nc, in_maps, core_ids=list(range(NCORES)), trace=trace)
    nshard = st["nshard"]
    outp = np.concatenate([res.results[c]["out"] for c in range(NCORES)], axis=0)
    out = np.empty_like(outp)
    out[perm] = outp
    return out.astype(np.float32), res


def kernel(x, edge_index, W1, b1, W2, b2):
    out, _ = _run(np.asarray(x, np.float32), np.asarray(edge_index),
                  np.asarray(W1, np.float32), np.asarray(b1, np.float32),
                  np.asarray(W2, np.float32), np.asarray(b2, np.float32))
    return out
